# revision 39
# baseline (speedup 1.0000x reference)
"""Trainium2 Bass kernel for nn_Graphs (soft decision-graph probability propagation).

Reference math (G=4 graphs, B=128 batch, N=255 internal nodes, L=256 leaves,
F=512 features, J=8 jumps):
  b  = sigmoid(x @ W_g^T + bias_g)                  (per graph: B x N)
  M0 = softmax(M_left, axis=dest), M1 = softmax(M_right, axis=dest)
  q  = [b*(M1-M0)+M0 | leaf-identity]               (per (g,batch): 511x511)
  prob <- q @ prob, J times, starting from e0; return leaf probs.

Restructure (v2, all-bf16 datapath):
  - q never materialized. With u = prob[internal], one jump is
      u' = E0 @ (r0*(1-b)*u) + E1 @ (r1*b*u)
    where E0/E1 are raw exp(M^T) tiles (bf16) and the softmax denominators
    r0/r1 are folded into the per-(node,batch) coefficients c0/c1.
  - Leaf rows only accumulate, and c0/c1 are jump-invariant, so the leaf
    block is hoisted out of the loop entirely:
      w = E0_leaf @ (sum_j c0*u_j) + E1_leaf @ (sum_j c1*u_j)
    The running sums (sacc) are maintained by gpsimd adds in the shadow of
    the PE jump stream; 4 leaf matmuls run once at the end.
  - Jump 0 is an outer product (u_0 = e0): 4 contract-dim-1 matmuls reading
    row 0 of E0/E1 against row 0 of the coefficients.
  - exp is one fused 1024-col ACT op per src tile (both matrices at once)
    with accum_out giving the combined row sum; a DVE half-reduce splits it
    into the two softmax denominators (r1 = recip(s01 - s_el)).
  - PE warm-up (HAM un-throttle) runs first and is chained INTO the real
    dependency graph (zj = 0*pwarm feeds the c01 coefficient ops, and two
    warm matmuls WAW-target the b-matmul psum), so the scheduler cannot
    push it to the end of the program (which is what happened in v1).

Sharding: 8 cores = (graph g = core//2) x (batch half h = core%2, 64 rows).
No cross-core communication. Host pre-transposes/pads/casts to bf16:
  - m2 (256,1024) bf16: M^T with source node on partitions; cols [0:512] =
    left matrix, [512:1024] = right; each 512 block = [internal 255 | NEG |
    leaf 256] (NEG pad -> exp = 0).
  - wxp (128,1280) bf16: per F-tile k, cols [320k:320k+256] = W_g^T block,
    [320k+256:320k+320] = x_half^T block.
  - biasp (128,2) f32: +bias/2 node-tiled (device computes b via
    tanh(0.5*logit + bias/2), same ACT table set as exp).
Output per core: (64,256) bf16 leaf-major; host assembles to (B,L,G) and
applies the reference interval clamp.
"""

import numpy as np
import ml_dtypes

G, B, N, L, F, J = 4, 128, 255, 256, 512, 8
BH = B // 2  # 64 batch rows per core
NCORES = 8
NEG = np.float32(-1e4)
BF16 = ml_dtypes.bfloat16

_CACHE = {}


def _build_program():
    import concourse.mybir as mybir
    from concourse import bacc
    from concourse.tile import TileContext

    f32 = mybir.dt.float32
    bf16 = mybir.dt.bfloat16
    AF = mybir.ActivationFunctionType
    AX = mybir.AxisListType
    mult = mybir.AluOpType.mult
    add = mybir.AluOpType.add

    nc = bacc.Bacc(None)
    p_m2 = nc.declare_dram_parameter("m2", [256, 1024], bf16, isOutput=False)
    p_wx = nc.declare_dram_parameter("wxp", [128, 1280], bf16, isOutput=False)
    p_bias = nc.declare_dram_parameter("biasp", [128, 2], f32, isOutput=False)
    p_out = nc.declare_dram_parameter("out", [BH, 256], bf16, isOutput=True)

    with TileContext(nc) as tc:
        with (
            tc.tile_pool(name="consts", bufs=1) as consts,
            tc.tile_pool(name="work", bufs=2) as work,
            tc.tile_pool(name="state", bufs=3) as state,
            tc.tile_pool(name="psum", bufs=2, space="PSUM") as psum,
            tc.tile_pool(name="psum_acc", bufs=1, space="PSUM") as psum_acc,
        ):
            # ---- DMA issue (first: these gate everything) ----
            # Each DMA trigger occupies its issuing engine ~0.65us, and each
            # transfer's completion semaphore lands ~1-1.5us after the data
            # (HBM receipt round-trip), so the 512KB m2 matrix goes as four
            # 128KB piece so exp of piece i overlaps the transfer+receipt of
            # piece i+1. t=0 pieces on HWDGE (sync), t=1 pieces + wxp + bias
            # on SWDGE (gpsimd) so the two trigger streams run in parallel.
            # ch0 (gates the first exps) alone on the sync queue; wxp first
            # on gpsimd so the b-matmuls unblock early, then ch1 and bias.
            # The two queues' transfers share the ~340GB/s SDMA aggregate.
            # Queue order = need order: ch0 gates the first exps, ch1 the
            # third, wxp only the tanh pair (via the b-matmuls), bias last.
            ch = [consts.tile([128, 1024], bf16, tag=f"ch{t}", name=f"ch{t}") for t in range(2)]
            nc.sync.dma_start(ch[0][:], p_m2[0:128, :])
            nc.gpsimd.dma_start(ch[1][:], p_m2[128:256, :])
            wx = consts.tile([128, 1280], bf16, tag="wx", name="wx")
            nc.sync.dma_start(wx[:], p_wx[:, :])
            bias = consts.tile([128, 2], f32, tag="bias", name="bias")
            nc.gpsimd.dma_start(bias[:], p_bias[:, :])

            # ---- PE warm-up (HAM un-throttle) ----
            # The jump loop's cadence includes the 8-matmul group latency,
            # which halves once the PE HAM un-throttles (needs ~3.4us of
            # sustained PE activity). Ten N=256 matmuls fill the otherwise
            # idle pre-loop PE window. The chain wsc/rsc memsets ->
            # pleaf-warms -> zw -> rsc-touch -> pq-warms -> (WAW) jump-0
            # anchors the stream in the real graph so the scheduler cannot
            # float it to the end of the program (v1's dead-code bug).
            wsc = consts.tile([128, 128], bf16, tag="wsc", name="wsc")
            rsc = consts.tile([128, 256], bf16, tag="rsc", name="rsc")
            nc.vector.memset(wsc[:], 0.0)
            nc.vector.memset(rsc[:], 0.0)
            pleaf = psum_acc.tile([BH, 256], f32, tag="pl", name="pl")
            pb = psum.tile([128, 2, BH], f32, tag="pb", name="pb")
            for _ in range(10):
                nc.tensor.matmul(pleaf[:], wsc[:, 0:BH], rsc[:], start=True, stop=True)
            zw = work.tile([BH, 1], f32, tag="zw", name="zw")
            nc.vector.tensor_scalar_mul(zw[:], pleaf[:, 0:1], 0.0)
            nc.vector.tensor_scalar_mul(rsc[0:1, 0:1], zw[0:1, :], 0.0)
            pq = [psum.tile([128, BH], f32, tag=f"pq{mt}", name=f"pq{mt}") for mt in range(2)]
            for mt in range(2):
                nc.tensor.matmul(pq[mt][:], wsc[:], rsc[:, 0:BH], start=True, stop=True)

            # ---- b = sigmoid(W @ x^T + bias) ----
            for mh in range(2):
                for k in range(4):
                    nc.tensor.matmul(
                        pb[:, mh, :],
                        wx[:, k * 320 + mh * 128:k * 320 + (mh + 1) * 128],
                        wx[:, k * 320 + 256:k * 320 + 320],
                        start=(k == 0), stop=(k == 3),
                    )

            # ---- exp of both matrices ----
            # Four 512-col slice-ops (ACT queue: el0, er0, el1, er1, eb0, eb1)
            # so each starts as soon as its chunk lands and each accum_out
            # yields that matrix-half's softmax row sum directly.
            elr = [consts.tile([128, 1024], bf16, tag=f"elr{t}", name=f"elr{t}") for t in range(2)]
            psl = [work.tile([128, 1], f32, tag=f"psl{t}", name=f"psl{t}") for t in range(2)]
            psr = [work.tile([128, 1], f32, tag=f"psr{t}", name=f"psr{t}") for t in range(2)]
            th = [work.tile([128, BH], f32, tag=f"th{mh}", name=f"th{mh}") for mh in range(2)]
            # ACT order [el0, er0, el1, th0, th1, er1]: the tanh pair slots
            # before the last exp so the t=0 coefficient chain (which gates
            # jump 0) is unblocked ~1us earlier. er1 uses accum_out for its
            # row sum so it doesn't queue behind the t=0 chain on the DVE;
            # the other three sums are DVE reduces (no ACT follower op).
            # b via tanh (same ACT table set as exp -> no second table load):
            # th = tanh((logit + bias)/2), so b = (1+th)/2, 1-b = (1-th)/2.
            nc.scalar.activation(elr[0][:, 0:512], ch[0][:, 0:512], AF.Exp)
            nc.scalar.activation(elr[0][:, 512:1024], ch[0][:, 512:1024], AF.Exp)
            nc.scalar.activation(elr[1][:, 0:512], ch[1][:, 0:512], AF.Exp)
            nc.scalar.activation(elr[1][:, 512:1024], ch[1][:, 512:1024], AF.Exp,
                                 accum_out=psr[1][:])
            for mh in range(2):
                nc.scalar.activation(th[mh][:], pb[:, mh, :], AF.Tanh,
                                     bias=bias[:, mh:mh + 1], scale=0.5)
            nc.vector.tensor_reduce(psl[0][:], elr[0][:, 0:512], axis=AX.XYZW, op=add)
            nc.vector.tensor_reduce(psr[0][:], elr[0][:, 512:1024], axis=AX.XYZW, op=add)
            nc.vector.tensor_reduce(psl[1][:], elr[1][:, 0:512], axis=AX.XYZW, op=add)

            # ---- softmax denominators + c01 coefficients (DVE) ----
            # c01[t][:,0,:] = r0*(1-b) = (1-th)/2 * r0
            # c01[t][:,1,:] = r1*b     = (1+th) * (r1/2)
            # with r0 = 1/psl, r1 = 1/psr. All single-source tensor_scalar
            # ops (fast DVE mode); per-partition scalars ride the AP operand.
            # Emission order = dependency-arrival order (DVE queue is FIFO):
            # the full t=0 chain first, then t=1 with its k0 half before the
            # er1-sum-dependent k1 half.
            c01 = [consts.tile([128, 2, BH], f32, tag=f"c01{t}", name=f"c01{t}") for t in range(2)]
            c01r0 = state.tile([1, 2, BH], bf16, tag="c01r0", name="c01r0")
            r0 = [work.tile([128, 1], f32, tag=f"r0_{t}", name=f"r0_{t}") for t in range(2)]
            r1h = [work.tile([128, 1], f32, tag=f"r1h_{t}", name=f"r1h_{t}") for t in range(2)]
            tb = [work.tile([128, BH], f32, tag=f"tb{t}", name=f"tb{t}") for t in range(2)]
            psr2 = [work.tile([128, 1], f32, tag=f"psr2_{t}", name=f"psr2_{t}") for t in range(2)]
            nc.vector.reciprocal(r0[0][:], psl[0][:])
            nc.vector.tensor_scalar_mul(psr2[0][:], psr[0][:], 2.0)
            nc.vector.reciprocal(r1h[0][:], psr2[0][:])
            nc.vector.tensor_scalar(
                c01[0][:, 1, :], th[0][:], 1.0, r1h[0][:], op0=add, op1=mult)
            nc.vector.tensor_scalar(tb[0][:], th[0][:], -0.5, 0.5, op0=mult, op1=add)
            nc.vector.tensor_scalar_mul(c01[0][:, 0, :], tb[0][:], r0[0][:])
            # bf16 seed state for jump 0 (= c01 row 0, since u_0 = e0)
            nc.vector.tensor_copy(c01r0[:], c01[0][0:1, :, :])
            nc.vector.reciprocal(r0[1][:], psl[1][:])
            nc.vector.tensor_scalar(tb[1][:], th[1][:], -0.5, 0.5, op0=mult, op1=add)
            nc.vector.tensor_scalar_mul(c01[1][:, 0, :], tb[1][:], r0[1][:])
            nc.vector.tensor_scalar_mul(psr2[1][:], psr[1][:], 2.0)
            nc.vector.reciprocal(r1h[1][:], psr2[1][:])
            nc.vector.tensor_scalar(
                c01[1][:, 1, :], th[1][:], 1.0, r1h[1][:], op0=add, op1=mult)


            # ---- leaf-sum accumulators (maintained by gpsimd) ----
            # sacc f32 through jump 6; jump 7's add writes the bf16 copy
            # directly (the cast the leaf matmuls need comes for free).
            sacc = [consts.tile([128, 2, BH], f32, tag=f"sacc{t}", name=f"sacc{t}") for t in range(2)]
            sacc_bf = [state.tile([128, 2, BH], bf16, tag=f"sbf{t}", name=f"sbf{t}") for t in range(2)]
            nc.gpsimd.memset(sacc[0][:], 0.0)
            nc.gpsimd.memset(sacc[1][:], 0.0)
            nc.gpsimd.tensor_add(sacc[0][0:1, :, :], sacc[0][0:1, :, :], c01r0[:])

            # ---- jump loop ----
            # Jump 0: u_1 = outer(E row0, c01 row0) -> 4 contract-1 matmuls.
            # Jumps 1..6: full rounds, 8 matmuls each, grouped by dest tile
            # so the next jump's DVE scale op starts after the first group.
            # Jump 7: scale+accumulate only (u_8 internal mass is unused).
            # (pq tiles were pre-allocated above as the warm-up WAW target.)
            for mt in range(2):
                ms = slice(mt * 128, (mt + 1) * 128)
                nc.tensor.matmul(pq[mt][:], elr[0][0:1, ms], c01r0[0:1, 0, :],
                                 start=True, stop=False)
                ms2 = slice(512 + mt * 128, 512 + (mt + 1) * 128)
                nc.tensor.matmul(pq[mt][:], elr[0][0:1, ms2], c01r0[0:1, 1, :],
                                 start=False, stop=True)

            for j in range(1, J):
                upv = [state.tile([128, 2, BH], bf16, tag=f"upv{t}", name=f"upv{t}") for t in range(2)]
                last = j == J - 1
                for t in range(2):
                    nc.vector.tensor_tensor(
                        out=upv[t][:], in0=c01[t][:],
                        in1=pq[t][:, None, :].broadcast_to([128, 2, BH]), op=mult)
                    # jump 7's scaled state skips the accumulator entirely
                    # (it feeds the leaf matmuls directly below); jump 6's
                    # add emits the bf16 copy the sacc leaf matmuls consume
                    if not last:
                        nc.gpsimd.tensor_add(
                            (sacc_bf if j == J - 2 else sacc)[t][:],
                            sacc[t][:], upv[t][:])
                if last:
                    upv7 = upv
                    break
                pq = [psum.tile([128, BH], f32, tag=f"pq{mt}", name=f"pq{mt}") for mt in range(2)]
                for mt in range(2):
                    ms = slice(mt * 128, (mt + 1) * 128)
                    ms2 = slice(512 + mt * 128, 512 + (mt + 1) * 128)
                    nc.tensor.matmul(pq[mt][:], elr[0][:, ms], upv[0][:, 0, :],
                                     start=True, stop=False)
                    nc.tensor.matmul(pq[mt][:], elr[0][:, ms2], upv[0][:, 1, :],
                                     start=False, stop=False)
                    # src tile 1 contracts [0:127]: row 127 is the pad node,
                    # which carries phantom mass under the u8 encoding
                    # (exp(0)=1 pad column) and must not propagate
                    nc.tensor.matmul(pq[mt][:], elr[1][0:127, ms], upv[1][0:127, 0, :],
                                     start=False, stop=False)
                    nc.tensor.matmul(pq[mt][:], elr[1][0:127, ms2], upv[1][0:127, 1, :],
                                     start=False, stop=True)

            # ---- leaf block (once; pleaf pre-allocated as warm-up target) ----
            # w = E_leaf^T @ sacc(j0..6) + E_leaf^T @ upv7: the sacc-based
            # matmuls start right after jump 6 (bf16 copy ready since j6's
            # add), warm; the upv7-based ones follow as the DVE finishes.
            nc.tensor.matmul(pleaf[:], sacc_bf[0][:, 0, :], elr[0][:, 256:512],
                             start=True, stop=False)
            nc.tensor.matmul(pleaf[:], sacc_bf[0][:, 1, :], elr[0][:, 768:1024],
                             start=False, stop=False)
            nc.tensor.matmul(pleaf[:], sacc_bf[1][0:127, 0, :], elr[1][0:127, 256:512],
                             start=False, stop=False)
            nc.tensor.matmul(pleaf[:], sacc_bf[1][0:127, 1, :], elr[1][0:127, 768:1024],
                             start=False, stop=False)
            nc.tensor.matmul(pleaf[:], upv7[0][:, 0, :], elr[0][:, 256:512],
                             start=False, stop=False)
            nc.tensor.matmul(pleaf[:], upv7[0][:, 1, :], elr[0][:, 768:1024],
                             start=False, stop=False)
            nc.tensor.matmul(pleaf[:], upv7[1][0:127, 0, :], elr[1][0:127, 256:512],
                             start=False, stop=False)
            nc.tensor.matmul(pleaf[:], upv7[1][0:127, 1, :], elr[1][0:127, 768:1024],
                             start=False, stop=True)

            # ---- output ----
            o = work.tile([BH, 256], bf16, tag="o", name="o")
            nc.vector.tensor_copy(o[:], pleaf[:])
            nc.sync.dma_start(p_out[:, :], o[:])

    nc.finalize()
    return nc


def _get_program():
    if "nc" not in _CACHE:
        _CACHE["nc"] = _build_program()
    return _CACHE["nc"]


def _prep_inputs(x, W, bias, M_left, M_right):
    """Host-side shard + layout prep. Core c -> graph c//2, batch half c%2."""
    in_maps = []
    m2_g, wt_g, bias_g = [], [], []
    for g in range(G):
        m2 = np.zeros((256, 1024), np.float32)
        tl = M_left[g].T  # (255, 511): src-major
        tr = M_right[g].T
        for base, src in ((0, tl), (512, tr)):
            m2[0:255, base:base + 255] = src[:, 0:255]
            m2[0:255, base + 256:base + 512] = src[:, 255:511]
            m2[0:255, base + 255] = NEG
        m2_g.append(m2.astype(BF16))
        wt = np.zeros((512, 256), np.float32)
        wt[:, 0:255] = W[g].T
        wt_g.append(wt)
        bp = np.zeros((256,), np.float32)
        bp[0:255] = bias[g] * 0.5
        bias_g.append(np.ascontiguousarray(bp.reshape(2, 128).T))  # (128, 2)
    xt_h = [np.ascontiguousarray(x[h * BH:(h + 1) * BH].T) for h in range(2)]
    for c in range(NCORES):
        g, h = c // 2, c % 2
        wxc = np.concatenate([wt_g[g], xt_h[h]], axis=1)  # (512, 320)
        wxp = np.ascontiguousarray(
            wxc.reshape(4, 128, 320).transpose(1, 0, 2).reshape(128, 1280)
        ).astype(BF16)
        in_maps.append({
            "m2": m2_g[g], "wxp": wxp, "biasp": bias_g[g],
        })
    return in_maps


def _assemble(results):
    eps = np.float32(1e-5)
    ret = np.empty((B, L, G), np.float32)
    for c in range(NCORES):
        g, h = c // 2, c % 2
        ret[h * BH:(h + 1) * BH, :, g] = results[c]["out"].astype(np.float32)
    ret = np.where(ret > 0.0, ret, eps)
    ret = np.where(ret < 1.0, ret, np.float32(1.0) - eps)
    return ret.astype(np.float32)


def run_on_device(in_maps, trace=False, **kw):
    from concourse.bass_utils import run_bass_kernel_spmd
    nc = _get_program()
    return run_bass_kernel_spmd(nc, in_maps, list(range(NCORES)), trace=trace, **kw)


def kernel(x, W, bias, M_left, M_right):
    in_maps = _prep_inputs(
        np.asarray(x, np.float32), np.asarray(W, np.float32),
        np.asarray(bias, np.float32), np.asarray(M_left, np.float32),
        np.asarray(M_right, np.float32),
    )
    res = run_on_device(in_maps)
    return _assemble(res.results)


# revision 41
# speedup vs baseline: 1.0573x; 1.0573x over previous
"""Trainium2 Bass kernel for nn_Graphs (soft decision-graph probability propagation).

Reference math (G=4 graphs, B=128 batch, N=255 internal nodes, L=256 leaves,
F=512 features, J=8 jumps):
  b  = sigmoid(x @ W_g^T + bias_g)                  (per graph: B x N)
  M0 = softmax(M_left, axis=dest), M1 = softmax(M_right, axis=dest)
  q  = [b*(M1-M0)+M0 | leaf-identity]               (per (g,batch): 511x511)
  prob <- q @ prob, J times, starting from e0; return leaf probs.

Restructure (v2, all-bf16 datapath):
  - q never materialized. With u = prob[internal], one jump is
      u' = E0 @ (r0*(1-b)*u) + E1 @ (r1*b*u)
    where E0/E1 are raw exp(M^T) tiles (bf16) and the softmax denominators
    r0/r1 are folded into the per-(node,batch) coefficients c0/c1.
  - Leaf rows only accumulate, and c0/c1 are jump-invariant, so the leaf
    block is hoisted out of the loop entirely:
      w = E0_leaf @ (sum_j c0*u_j) + E1_leaf @ (sum_j c1*u_j)
    The running sums (sacc) are maintained by gpsimd adds in the shadow of
    the PE jump stream; 4 leaf matmuls run once at the end.
  - Jump 0 is an outer product (u_0 = e0): 4 contract-dim-1 matmuls reading
    row 0 of E0/E1 against row 0 of the coefficients.
  - exp is one fused 1024-col ACT op per src tile (both matrices at once)
    with accum_out giving the combined row sum; a DVE half-reduce splits it
    into the two softmax denominators (r1 = recip(s01 - s_el)).
  - PE warm-up (HAM un-throttle) runs first and is chained INTO the real
    dependency graph (zj = 0*pwarm feeds the c01 coefficient ops, and two
    warm matmuls WAW-target the b-matmul psum), so the scheduler cannot
    push it to the end of the program (which is what happened in v1).

Sharding: 8 cores = (graph g = core//2) x (batch half h = core%2, 64 rows).
No cross-core communication. Host pre-transposes/pads/casts to bf16:
  - m2 (256,1024) bf16: M^T with source node on partitions; cols [0:512] =
    left matrix, [512:1024] = right; each 512 block = [internal 255 | NEG |
    leaf 256] (NEG pad -> exp = 0).
  - wxp (128,1280) bf16: per F-tile k, cols [320k:320k+256] = W_g^T block,
    [320k+256:320k+320] = x_half^T block.
  - biasp (128,2) f32: +bias/2 node-tiled (device computes b via
    tanh(0.5*logit + bias/2), same ACT table set as exp).
Output per core: (64,256) bf16 leaf-major; host assembles to (B,L,G) and
applies the reference interval clamp.
"""

import numpy as np
import ml_dtypes

G, B, N, L, F, J = 4, 128, 255, 256, 512, 8
BH = B // 2  # 64 batch rows per core
NCORES = 8
NEG = np.float32(-1e4)
BF16 = ml_dtypes.bfloat16

_CACHE = {}


def _build_program():
    import concourse.mybir as mybir
    from concourse import bacc
    from concourse.tile import TileContext

    f32 = mybir.dt.float32
    bf16 = mybir.dt.bfloat16
    AF = mybir.ActivationFunctionType
    AX = mybir.AxisListType
    mult = mybir.AluOpType.mult
    add = mybir.AluOpType.add

    nc = bacc.Bacc(None)
    p_m2 = nc.declare_dram_parameter("m2", [256, 1024], bf16, isOutput=False)
    p_wx = nc.declare_dram_parameter("wxp", [128, 1280], bf16, isOutput=False)
    p_bias = nc.declare_dram_parameter("biasp", [128, 2], f32, isOutput=False)
    p_out = nc.declare_dram_parameter("out", [BH, 256], bf16, isOutput=True)

    with TileContext(nc) as tc:
        with (
            tc.tile_pool(name="consts", bufs=1) as consts,
            tc.tile_pool(name="work", bufs=2) as work,
            tc.tile_pool(name="state", bufs=3) as state,
            tc.tile_pool(name="psum", bufs=2, space="PSUM") as psum,
            tc.tile_pool(name="psum_acc", bufs=1, space="PSUM") as psum_acc,
        ):
            # ---- DMA issue (first: these gate everything) ----
            # Each DMA trigger occupies its issuing engine ~0.65us, and each
            # transfer's completion semaphore lands ~1-1.5us after the data
            # (HBM receipt round-trip), so the 512KB m2 matrix goes as four
            # 128KB piece so exp of piece i overlaps the transfer+receipt of
            # piece i+1. t=0 pieces on HWDGE (sync), t=1 pieces + wxp + bias
            # on SWDGE (gpsimd) so the two trigger streams run in parallel.
            # ch0 (gates the first exps) alone on the sync queue; wxp first
            # on gpsimd so the b-matmuls unblock early, then ch1 and bias.
            # The two queues' transfers share the ~340GB/s SDMA aggregate.
            # Queue order = need order: ch0 gates the first exps, ch1 the
            # third, wxp only the tanh pair (via the b-matmuls), bias last.
            ch = [consts.tile([128, 1024], bf16, tag=f"ch{t}", name=f"ch{t}") for t in range(2)]
            nc.sync.dma_start(ch[0][:], p_m2[0:128, :])
            nc.gpsimd.dma_start(ch[1][:], p_m2[128:256, :])
            wx = consts.tile([128, 1280], bf16, tag="wx", name="wx")
            nc.sync.dma_start(wx[:], p_wx[:, :])
            bias = consts.tile([128, 2], f32, tag="bias", name="bias")
            nc.gpsimd.dma_start(bias[:], p_bias[:, :])

            # ---- PE warm-up (HAM un-throttle) ----
            # The jump loop's cadence includes the 8-matmul group latency,
            # which halves once the PE HAM un-throttles (needs ~3.4us of
            # sustained PE activity). Ten N=256 matmuls fill the otherwise
            # idle pre-loop PE window. The chain wsc/rsc memsets ->
            # pleaf-warms -> zw -> rsc-touch -> pq-warms -> (WAW) jump-0
            # anchors the stream in the real graph so the scheduler cannot
            # float it to the end of the program (v1's dead-code bug).
            wsc = consts.tile([128, 128], bf16, tag="wsc", name="wsc")
            rsc = consts.tile([128, 256], bf16, tag="rsc", name="rsc")
            nc.vector.memset(wsc[:], 0.0)
            nc.vector.memset(rsc[:], 0.0)
            pleaf = psum_acc.tile([BH, 256], f32, tag="pl", name="pl")
            pb = psum.tile([128, 2, BH], f32, tag="pb", name="pb")
            for _ in range(10):
                nc.tensor.matmul(pleaf[:], wsc[:, 0:BH], rsc[:], start=True, stop=True)
            zw = work.tile([BH, 1], f32, tag="zw", name="zw")
            nc.vector.tensor_scalar_mul(zw[:], pleaf[:, 0:1], 0.0)
            nc.vector.tensor_scalar_mul(rsc[0:1, 0:1], zw[0:1, :], 0.0)
            pq = [psum.tile([128, BH], f32, tag=f"pq{mt}", name=f"pq{mt}") for mt in range(2)]
            for mt in range(2):
                nc.tensor.matmul(pq[mt][:], wsc[:], rsc[:, 0:BH], start=True, stop=True)

            # ---- b = sigmoid(W @ x^T + bias) ----
            for mh in range(2):
                for k in range(4):
                    nc.tensor.matmul(
                        pb[:, mh, :],
                        wx[:, k * 320 + mh * 128:k * 320 + (mh + 1) * 128],
                        wx[:, k * 320 + 256:k * 320 + 320],
                        start=(k == 0), stop=(k == 3),
                    )

            # ---- exp of both matrices ----
            # Four 512-col slice-ops (ACT queue: el0, er0, el1, er1, eb0, eb1)
            # so each starts as soon as its chunk lands and each accum_out
            # yields that matrix-half's softmax row sum directly.
            elr = [consts.tile([128, 1024], bf16, tag=f"elr{t}", name=f"elr{t}") for t in range(2)]
            psl = [work.tile([128, 1], f32, tag=f"psl{t}", name=f"psl{t}") for t in range(2)]
            psr = [work.tile([128, 1], f32, tag=f"psr{t}", name=f"psr{t}") for t in range(2)]
            th = [work.tile([128, BH], f32, tag=f"th{mh}", name=f"th{mh}") for mh in range(2)]
            # ACT order [el0, er0, el1, th0, th1, er1]: the tanh pair slots
            # before the last exp so the t=0 coefficient chain (which gates
            # jump 0) is unblocked ~1us earlier. er1 uses accum_out for its
            # row sum so it doesn't queue behind the t=0 chain on the DVE;
            # the other three sums are DVE reduces (no ACT follower op).
            # b via tanh (same ACT table set as exp -> no second table load):
            # th = tanh((logit + bias)/2), so b = (1+th)/2, 1-b = (1-th)/2.
            nc.scalar.activation(elr[0][:, 0:512], ch[0][:, 0:512], AF.Exp)
            nc.scalar.activation(elr[0][:, 512:1024], ch[0][:, 512:1024], AF.Exp)
            nc.scalar.activation(elr[1][:, 0:512], ch[1][:, 0:512], AF.Exp)
            nc.scalar.activation(elr[1][:, 512:1024], ch[1][:, 512:1024], AF.Exp,
                                 accum_out=psr[1][:])
            for mh in range(2):
                nc.scalar.activation(th[mh][:], pb[:, mh, :], AF.Tanh,
                                     bias=bias[:, mh:mh + 1], scale=0.5)
            nc.vector.tensor_reduce(psl[0][:], elr[0][:, 0:512], axis=AX.XYZW, op=add)
            nc.vector.tensor_reduce(psr[0][:], elr[0][:, 512:1024], axis=AX.XYZW, op=add)
            nc.vector.tensor_reduce(psl[1][:], elr[1][:, 0:512], axis=AX.XYZW, op=add)

            # ---- softmax denominators + c01 coefficients (DVE) ----
            # c01[t][:,0,:] = r0*(1-b) = (1-th)/2 * r0
            # c01[t][:,1,:] = r1*b     = (1+th) * (r1/2)
            # with r0 = 1/psl, r1 = 1/psr. All single-source tensor_scalar
            # ops (fast DVE mode); per-partition scalars ride the AP operand.
            # Emission order = dependency-arrival order (DVE queue is FIFO):
            # the full t=0 chain first, then t=1 with its k0 half before the
            # er1-sum-dependent k1 half.
            c01 = [consts.tile([128, 2, BH], f32, tag=f"c01{t}", name=f"c01{t}") for t in range(2)]
            c01r0 = state.tile([1, 2, BH], bf16, tag="c01r0", name="c01r0")
            r0 = [work.tile([128, 1], f32, tag=f"r0_{t}", name=f"r0_{t}") for t in range(2)]
            r1h = [work.tile([128, 1], f32, tag=f"r1h_{t}", name=f"r1h_{t}") for t in range(2)]
            tb = [work.tile([128, BH], f32, tag=f"tb{t}", name=f"tb{t}") for t in range(2)]
            psr2 = [work.tile([128, 1], f32, tag=f"psr2_{t}", name=f"psr2_{t}") for t in range(2)]
            nc.vector.reciprocal(r0[0][:], psl[0][:])
            nc.vector.tensor_scalar_mul(psr2[0][:], psr[0][:], 2.0)
            nc.vector.reciprocal(r1h[0][:], psr2[0][:])
            nc.vector.tensor_scalar(
                c01[0][:, 1, :], th[0][:], 1.0, r1h[0][:], op0=add, op1=mult)
            nc.vector.tensor_scalar(tb[0][:], th[0][:], -0.5, 0.5, op0=mult, op1=add)
            nc.vector.tensor_scalar_mul(c01[0][:, 0, :], tb[0][:], r0[0][:])
            # bf16 seed state for jump 0 (= c01 row 0, since u_0 = e0)
            nc.vector.tensor_copy(c01r0[:], c01[0][0:1, :, :])
            nc.vector.reciprocal(r0[1][:], psl[1][:])
            nc.vector.tensor_scalar(tb[1][:], th[1][:], -0.5, 0.5, op0=mult, op1=add)
            nc.vector.tensor_scalar_mul(c01[1][:, 0, :], tb[1][:], r0[1][:])
            nc.vector.tensor_scalar_mul(psr2[1][:], psr[1][:], 2.0)
            nc.vector.reciprocal(r1h[1][:], psr2[1][:])
            nc.vector.tensor_scalar(
                c01[1][:, 1, :], th[1][:], 1.0, r1h[1][:], op0=add, op1=mult)


            # ---- leaf-sum accumulators (maintained by gpsimd) ----
            # sacc f32 through jump 6; jump 7's add writes the bf16 copy
            # directly (the cast the leaf matmuls need comes for free).
            sacc = [consts.tile([128, 2, BH], f32, tag=f"sacc{t}", name=f"sacc{t}") for t in range(2)]
            sacc_bf = [state.tile([128, 2, BH], bf16, tag=f"sbf{t}", name=f"sbf{t}") for t in range(2)]
            nc.gpsimd.memset(sacc[0][:], 0.0)
            nc.gpsimd.memset(sacc[1][:], 0.0)
            nc.gpsimd.tensor_add(sacc[0][0:1, :, :], sacc[0][0:1, :, :], c01r0[:])

            # ---- jump loop ----
            # Jump 0: u_1 = outer(E row0, c01 row0) -> 4 contract-1 matmuls.
            # Jumps 1..6: full rounds, 8 matmuls each, grouped by dest tile
            # so the next jump's DVE scale op starts after the first group.
            # Jump 7: scale+accumulate only (u_8 internal mass is unused).
            # (pq tiles were pre-allocated above as the warm-up WAW target.)
            for mt in range(2):
                ms = slice(mt * 128, (mt + 1) * 128)
                nc.tensor.matmul(pq[mt][:], elr[0][0:1, ms], c01r0[0:1, 0, :],
                                 start=True, stop=False)
                ms2 = slice(512 + mt * 128, 512 + (mt + 1) * 128)
                nc.tensor.matmul(pq[mt][:], elr[0][0:1, ms2], c01r0[0:1, 1, :],
                                 start=False, stop=True)

            for j in range(1, J):
                upv = [state.tile([128, 2, BH], bf16, tag=f"upv{t}", name=f"upv{t}") for t in range(2)]
                last = j == J - 1
                for t in range(2):
                    nc.vector.tensor_tensor(
                        out=upv[t][:], in0=c01[t][:],
                        in1=pq[t][:, None, :].broadcast_to([128, 2, BH]), op=mult)
                    # jumps 1-6 accumulate on gpsimd (in the PE/DVE shadow);
                    # jump 7's final adds ride the DVE instead — the gpsimd
                    # queue drains ~1.5us behind the loop, and these two ops
                    # gate the leaf block. The bf16 output doubles as the
                    # cast the leaf matmuls need.
                    if last:
                        nc.vector.tensor_add(sacc_bf[t][:], sacc[t][:], upv[t][:])
                    else:
                        nc.gpsimd.tensor_add(sacc[t][:], sacc[t][:], upv[t][:])
                if last:
                    break
                pq = [psum.tile([128, BH], f32, tag=f"pq{mt}", name=f"pq{mt}") for mt in range(2)]
                for mt in range(2):
                    ms = slice(mt * 128, (mt + 1) * 128)
                    ms2 = slice(512 + mt * 128, 512 + (mt + 1) * 128)
                    nc.tensor.matmul(pq[mt][:], elr[0][:, ms], upv[0][:, 0, :],
                                     start=True, stop=False)
                    nc.tensor.matmul(pq[mt][:], elr[0][:, ms2], upv[0][:, 1, :],
                                     start=False, stop=False)
                    # src tile 1 contracts [0:127]: row 127 is the pad node,
                    # which carries phantom mass under the u8 encoding
                    # (exp(0)=1 pad column) and must not propagate
                    nc.tensor.matmul(pq[mt][:], elr[1][0:127, ms], upv[1][0:127, 0, :],
                                     start=False, stop=False)
                    nc.tensor.matmul(pq[mt][:], elr[1][0:127, ms2], upv[1][0:127, 1, :],
                                     start=False, stop=True)

            # ---- leaf block (once; pleaf pre-allocated as warm-up target) ----
            nc.tensor.matmul(pleaf[:], sacc_bf[0][:, 0, :], elr[0][:, 256:512],
                             start=True, stop=False)
            nc.tensor.matmul(pleaf[:], sacc_bf[0][:, 1, :], elr[0][:, 768:1024],
                             start=False, stop=False)
            nc.tensor.matmul(pleaf[:], sacc_bf[1][0:127, 0, :], elr[1][0:127, 256:512],
                             start=False, stop=False)
            nc.tensor.matmul(pleaf[:], sacc_bf[1][0:127, 1, :], elr[1][0:127, 768:1024],
                             start=False, stop=True)

            # ---- output ----
            o = work.tile([BH, 256], bf16, tag="o", name="o")
            nc.vector.tensor_copy(o[:], pleaf[:])
            nc.sync.dma_start(p_out[:, :], o[:])

    nc.finalize()
    return nc


def _get_program():
    if "nc" not in _CACHE:
        _CACHE["nc"] = _build_program()
    return _CACHE["nc"]


def _prep_inputs(x, W, bias, M_left, M_right):
    """Host-side shard + layout prep. Core c -> graph c//2, batch half c%2."""
    in_maps = []
    m2_g, wt_g, bias_g = [], [], []
    for g in range(G):
        m2 = np.zeros((256, 1024), np.float32)
        tl = M_left[g].T  # (255, 511): src-major
        tr = M_right[g].T
        for base, src in ((0, tl), (512, tr)):
            m2[0:255, base:base + 255] = src[:, 0:255]
            m2[0:255, base + 256:base + 512] = src[:, 255:511]
            m2[0:255, base + 255] = NEG
        m2_g.append(m2.astype(BF16))
        wt = np.zeros((512, 256), np.float32)
        wt[:, 0:255] = W[g].T
        wt_g.append(wt)
        bp = np.zeros((256,), np.float32)
        bp[0:255] = bias[g] * 0.5
        bias_g.append(np.ascontiguousarray(bp.reshape(2, 128).T))  # (128, 2)
    xt_h = [np.ascontiguousarray(x[h * BH:(h + 1) * BH].T) for h in range(2)]
    for c in range(NCORES):
        g, h = c // 2, c % 2
        wxc = np.concatenate([wt_g[g], xt_h[h]], axis=1)  # (512, 320)
        wxp = np.ascontiguousarray(
            wxc.reshape(4, 128, 320).transpose(1, 0, 2).reshape(128, 1280)
        ).astype(BF16)
        in_maps.append({
            "m2": m2_g[g], "wxp": wxp, "biasp": bias_g[g],
        })
    return in_maps


def _assemble(results):
    eps = np.float32(1e-5)
    ret = np.empty((B, L, G), np.float32)
    for c in range(NCORES):
        g, h = c // 2, c % 2
        ret[h * BH:(h + 1) * BH, :, g] = results[c]["out"].astype(np.float32)
    ret = np.where(ret > 0.0, ret, eps)
    ret = np.where(ret < 1.0, ret, np.float32(1.0) - eps)
    return ret.astype(np.float32)


def run_on_device(in_maps, trace=False, **kw):
    from concourse.bass_utils import run_bass_kernel_spmd
    nc = _get_program()
    return run_bass_kernel_spmd(nc, in_maps, list(range(NCORES)), trace=trace, **kw)


def kernel(x, W, bias, M_left, M_right):
    in_maps = _prep_inputs(
        np.asarray(x, np.float32), np.asarray(W, np.float32),
        np.asarray(bias, np.float32), np.asarray(M_left, np.float32),
        np.asarray(M_right, np.float32),
    )
    res = run_on_device(in_maps)
    return _assemble(res.results)


# revision 42
# speedup vs baseline: 1.0616x; 1.0040x over previous
"""Trainium2 Bass kernel for nn_Graphs (soft decision-graph probability propagation).

Reference math (G=4 graphs, B=128 batch, N=255 internal nodes, L=256 leaves,
F=512 features, J=8 jumps):
  b  = sigmoid(x @ W_g^T + bias_g)                  (per graph: B x N)
  M0 = softmax(M_left, axis=dest), M1 = softmax(M_right, axis=dest)
  q  = [b*(M1-M0)+M0 | leaf-identity]               (per (g,batch): 511x511)
  prob <- q @ prob, J times, starting from e0; return leaf probs.

Restructure (v2, all-bf16 datapath):
  - q never materialized. With u = prob[internal], one jump is
      u' = E0 @ (r0*(1-b)*u) + E1 @ (r1*b*u)
    where E0/E1 are raw exp(M^T) tiles (bf16) and the softmax denominators
    r0/r1 are folded into the per-(node,batch) coefficients c0/c1.
  - Leaf rows only accumulate, and c0/c1 are jump-invariant, so the leaf
    block is hoisted out of the loop entirely:
      w = E0_leaf @ (sum_j c0*u_j) + E1_leaf @ (sum_j c1*u_j)
    The running sums (sacc) are maintained by gpsimd adds in the shadow of
    the PE jump stream; 4 leaf matmuls run once at the end.
  - Jump 0 is an outer product (u_0 = e0): 4 contract-dim-1 matmuls reading
    row 0 of E0/E1 against row 0 of the coefficients.
  - exp is one fused 1024-col ACT op per src tile (both matrices at once)
    with accum_out giving the combined row sum; a DVE half-reduce splits it
    into the two softmax denominators (r1 = recip(s01 - s_el)).
  - PE warm-up (HAM un-throttle) runs first and is chained INTO the real
    dependency graph (zj = 0*pwarm feeds the c01 coefficient ops, and two
    warm matmuls WAW-target the b-matmul psum), so the scheduler cannot
    push it to the end of the program (which is what happened in v1).

Sharding: 8 cores = (graph g = core//2) x (batch half h = core%2, 64 rows).
No cross-core communication. Host pre-transposes/pads/casts to bf16:
  - m2 (256,1024) bf16: M^T with source node on partitions; cols [0:512] =
    left matrix, [512:1024] = right; each 512 block = [internal 255 | NEG |
    leaf 256] (NEG pad -> exp = 0).
  - wxp (128,1280) bf16: per F-tile k, cols [320k:320k+256] = W_g^T block,
    [320k+256:320k+320] = x_half^T block.
  - biasp (128,2) f32: +bias/2 node-tiled (device computes b via
    tanh(0.5*logit + bias/2), same ACT table set as exp).
Output per core: (64,256) bf16 leaf-major; host assembles to (B,L,G) and
applies the reference interval clamp.
"""

import numpy as np
import ml_dtypes

G, B, N, L, F, J = 4, 128, 255, 256, 512, 8
BH = B // 2  # 64 batch rows per core
NCORES = 8
NEG = np.float32(-1e4)
BF16 = ml_dtypes.bfloat16

_CACHE = {}


def _build_program():
    import concourse.mybir as mybir
    from concourse import bacc
    from concourse.tile import TileContext

    f32 = mybir.dt.float32
    bf16 = mybir.dt.bfloat16
    AF = mybir.ActivationFunctionType
    AX = mybir.AxisListType
    mult = mybir.AluOpType.mult
    add = mybir.AluOpType.add

    nc = bacc.Bacc(None)
    p_m2 = nc.declare_dram_parameter("m2", [256, 1024], bf16, isOutput=False)
    p_wx = nc.declare_dram_parameter("wxp", [128, 1280], bf16, isOutput=False)
    p_bias = nc.declare_dram_parameter("biasp", [128, 2], f32, isOutput=False)
    p_out = nc.declare_dram_parameter("out", [BH, 256], bf16, isOutput=True)

    with TileContext(nc) as tc:
        with (
            tc.tile_pool(name="consts", bufs=1) as consts,
            tc.tile_pool(name="work", bufs=2) as work,
            tc.tile_pool(name="state", bufs=3) as state,
            tc.tile_pool(name="psum", bufs=2, space="PSUM") as psum,
            tc.tile_pool(name="psum_acc", bufs=1, space="PSUM") as psum_acc,
        ):
            # ---- DMA issue (first: these gate everything) ----
            # Each DMA trigger occupies its issuing engine ~0.65us, and each
            # transfer's completion semaphore lands ~1-1.5us after the data
            # (HBM receipt round-trip), so the 512KB m2 matrix goes as four
            # 128KB piece so exp of piece i overlaps the transfer+receipt of
            # piece i+1. t=0 pieces on HWDGE (sync), t=1 pieces + wxp + bias
            # on SWDGE (gpsimd) so the two trigger streams run in parallel.
            # ch0 (gates the first exps) alone on the sync queue; wxp first
            # on gpsimd so the b-matmuls unblock early, then ch1 and bias.
            # The two queues' transfers share the ~340GB/s SDMA aggregate.
            # Queue order = need order: ch0 gates the first exps, ch1 the
            # third, wxp only the tanh pair (via the b-matmuls), bias last.
            ch = [consts.tile([128, 1024], bf16, tag=f"ch{t}", name=f"ch{t}") for t in range(2)]
            nc.sync.dma_start(ch[0][:], p_m2[0:128, :])
            nc.gpsimd.dma_start(ch[1][:], p_m2[128:256, :])
            wx = consts.tile([128, 1280], bf16, tag="wx", name="wx")
            nc.sync.dma_start(wx[:], p_wx[:, :])
            bias = consts.tile([128, 2], f32, tag="bias", name="bias")
            nc.gpsimd.dma_start(bias[:], p_bias[:, :])

            # ---- PE warm-up (HAM un-throttle) ----
            # The jump loop's cadence includes the 8-matmul group latency,
            # which halves once the PE HAM un-throttles (needs ~3.4us of
            # sustained PE activity). Ten N=256 matmuls fill the otherwise
            # idle pre-loop PE window. The chain wsc/rsc memsets ->
            # pleaf-warms -> zw -> rsc-touch -> pq-warms -> (WAW) jump-0
            # anchors the stream in the real graph so the scheduler cannot
            # float it to the end of the program (v1's dead-code bug).
            wsc = consts.tile([128, 128], bf16, tag="wsc", name="wsc")
            rsc = consts.tile([128, 256], bf16, tag="rsc", name="rsc")
            nc.vector.memset(wsc[:], 0.0)
            nc.vector.memset(rsc[:], 0.0)
            pleaf = psum_acc.tile([BH, 256], f32, tag="pl", name="pl")
            pb = psum.tile([128, 2, BH], f32, tag="pb", name="pb")
            for _ in range(10):
                nc.tensor.matmul(pleaf[:], wsc[:, 0:BH], rsc[:], start=True, stop=True)
            zw = work.tile([BH, 1], f32, tag="zw", name="zw")
            nc.vector.tensor_scalar_mul(zw[:], pleaf[:, 0:1], 0.0)
            nc.vector.tensor_scalar_mul(rsc[0:1, 0:1], zw[0:1, :], 0.0)
            pq = [psum.tile([128, BH], f32, tag=f"pq{mt}", name=f"pq{mt}") for mt in range(2)]
            for mt in range(2):
                nc.tensor.matmul(pq[mt][:], wsc[:], rsc[:, 0:BH], start=True, stop=True)

            # ---- b = sigmoid(W @ x^T + bias) ----
            for mh in range(2):
                for k in range(4):
                    nc.tensor.matmul(
                        pb[:, mh, :],
                        wx[:, k * 320 + mh * 128:k * 320 + (mh + 1) * 128],
                        wx[:, k * 320 + 256:k * 320 + 320],
                        start=(k == 0), stop=(k == 3),
                    )

            # ---- exp of both matrices ----
            # Four 512-col slice-ops (ACT queue: el0, er0, el1, er1, eb0, eb1)
            # so each starts as soon as its chunk lands and each accum_out
            # yields that matrix-half's softmax row sum directly.
            elr = [consts.tile([128, 1024], bf16, tag=f"elr{t}", name=f"elr{t}") for t in range(2)]
            psl = [work.tile([128, 1], f32, tag=f"psl{t}", name=f"psl{t}") for t in range(2)]
            psr = [work.tile([128, 1], f32, tag=f"psr{t}", name=f"psr{t}") for t in range(2)]
            th = [work.tile([128, BH], f32, tag=f"th{mh}", name=f"th{mh}") for mh in range(2)]
            # ACT order [el0, er0, el1, th0, th1, er1]: the tanh pair slots
            # before the last exp so the t=0 coefficient chain (which gates
            # jump 0) is unblocked ~1us earlier. er1 uses accum_out for its
            # row sum so it doesn't queue behind the t=0 chain on the DVE;
            # the other three sums are DVE reduces (no ACT follower op).
            # b via tanh (same ACT table set as exp -> no second table load):
            # th = tanh((logit + bias)/2), so b = (1+th)/2, 1-b = (1-th)/2.
            nc.scalar.activation(elr[0][:, 0:512], ch[0][:, 0:512], AF.Exp)
            nc.scalar.activation(elr[0][:, 512:1024], ch[0][:, 512:1024], AF.Exp)
            nc.scalar.activation(elr[1][:, 0:512], ch[1][:, 0:512], AF.Exp)
            nc.scalar.activation(elr[1][:, 512:1024], ch[1][:, 512:1024], AF.Exp,
                                 accum_out=psr[1][:])
            for mh in range(2):
                nc.scalar.activation(th[mh][:], pb[:, mh, :], AF.Tanh,
                                     bias=bias[:, mh:mh + 1], scale=0.5)
            nc.vector.tensor_reduce(psl[0][:], elr[0][:, 0:512], axis=AX.XYZW, op=add)
            nc.vector.tensor_reduce(psr[0][:], elr[0][:, 512:1024], axis=AX.XYZW, op=add)
            nc.vector.tensor_reduce(psl[1][:], elr[1][:, 0:512], axis=AX.XYZW, op=add)

            # ---- softmax denominators + c01 coefficients (DVE) ----
            # c01[t][:,0,:] = r0*(1-b) = (1-th)/2 * r0
            # c01[t][:,1,:] = r1*b     = (1+th) * (r1/2)
            # with r0 = 1/psl, r1 = 1/psr. All single-source tensor_scalar
            # ops (fast DVE mode); per-partition scalars ride the AP operand.
            # Emission order = dependency-arrival order (DVE queue is FIFO):
            # the full t=0 chain first, then t=1 with its k0 half before the
            # er1-sum-dependent k1 half.
            c01 = [consts.tile([128, 2, BH], f32, tag=f"c01{t}", name=f"c01{t}") for t in range(2)]
            c01r0 = state.tile([1, 2, BH], bf16, tag="c01r0", name="c01r0")
            r0 = [work.tile([128, 1], f32, tag=f"r0_{t}", name=f"r0_{t}") for t in range(2)]
            r1h = [work.tile([128, 1], f32, tag=f"r1h_{t}", name=f"r1h_{t}") for t in range(2)]
            tb = [work.tile([128, BH], f32, tag=f"tb{t}", name=f"tb{t}") for t in range(2)]
            psr2 = [work.tile([128, 1], f32, tag=f"psr2_{t}", name=f"psr2_{t}") for t in range(2)]
            nc.vector.reciprocal(r0[0][:], psl[0][:])
            nc.vector.tensor_scalar_mul(psr2[0][:], psr[0][:], 2.0)
            nc.vector.reciprocal(r1h[0][:], psr2[0][:])
            nc.vector.tensor_scalar(
                c01[0][:, 1, :], th[0][:], 1.0, r1h[0][:], op0=add, op1=mult)
            nc.vector.tensor_scalar(tb[0][:], th[0][:], -0.5, 0.5, op0=mult, op1=add)
            nc.vector.tensor_scalar_mul(c01[0][:, 0, :], tb[0][:], r0[0][:])
            # bf16 seed state for jump 0 (= c01 row 0, since u_0 = e0)
            nc.vector.tensor_copy(c01r0[:], c01[0][0:1, :, :])
            nc.vector.reciprocal(r0[1][:], psl[1][:])
            nc.vector.tensor_scalar(tb[1][:], th[1][:], -0.5, 0.5, op0=mult, op1=add)
            nc.vector.tensor_scalar_mul(c01[1][:, 0, :], tb[1][:], r0[1][:])
            nc.vector.tensor_scalar_mul(psr2[1][:], psr[1][:], 2.0)
            nc.vector.reciprocal(r1h[1][:], psr2[1][:])
            nc.vector.tensor_scalar(
                c01[1][:, 1, :], th[1][:], 1.0, r1h[1][:], op0=add, op1=mult)


            # ---- leaf-sum accumulators (maintained by gpsimd) ----
            # sacc f32 through jump 6; jump 7's add writes the bf16 copy
            # directly (the cast the leaf matmuls need comes for free).
            sacc = [consts.tile([128, 2, BH], f32, tag=f"sacc{t}", name=f"sacc{t}") for t in range(2)]
            sacc_bf = [state.tile([128, 2, BH], bf16, tag=f"sbf{t}", name=f"sbf{t}") for t in range(2)]
            nc.gpsimd.memset(sacc[0][:], 0.0)
            nc.gpsimd.memset(sacc[1][:], 0.0)
            nc.gpsimd.tensor_add(sacc[0][0:1, :, :], sacc[0][0:1, :, :], c01r0[:])

            # ---- jump loop ----
            # Jump 0: u_1 = outer(E row0, c01 row0) -> 4 contract-1 matmuls.
            # Jumps 1..6: full rounds, 8 matmuls each, grouped by dest tile
            # so the next jump's DVE scale op starts after the first group.
            # Jump 7: scale+accumulate only (u_8 internal mass is unused).
            # (pq tiles were pre-allocated above as the warm-up WAW target.)
            for mt in range(2):
                ms = slice(mt * 128, (mt + 1) * 128)
                nc.tensor.matmul(pq[mt][:], elr[0][0:1, ms], c01r0[0:1, 0, :],
                                 start=True, stop=False)
                ms2 = slice(512 + mt * 128, 512 + (mt + 1) * 128)
                nc.tensor.matmul(pq[mt][:], elr[0][0:1, ms2], c01r0[0:1, 1, :],
                                 start=False, stop=True)

            for j in range(1, J):
                upv = [state.tile([128, 2, BH], bf16, tag=f"upv{t}", name=f"upv{t}") for t in range(2)]
                last = j == J - 1
                for t in range(2):
                    nc.vector.tensor_tensor(
                        out=upv[t][:], in0=c01[t][:],
                        in1=pq[t][:, None, :].broadcast_to([128, 2, BH]), op=mult)
                    # final add emits the bf16 copy the leaf matmuls consume
                    # (measured: keeping all adds on gpsimd beats moving the
                    # last pair to DVE or feeding upv7 to extra leaf matmuls
                    # — both trailing queues lag the loop by ~1.5us)
                    nc.gpsimd.tensor_add(
                        (sacc_bf if last else sacc)[t][:], sacc[t][:], upv[t][:])
                if last:
                    break
                pq = [psum.tile([128, BH], f32, tag=f"pq{mt}", name=f"pq{mt}") for mt in range(2)]
                for mt in range(2):
                    ms = slice(mt * 128, (mt + 1) * 128)
                    ms2 = slice(512 + mt * 128, 512 + (mt + 1) * 128)
                    nc.tensor.matmul(pq[mt][:], elr[0][:, ms], upv[0][:, 0, :],
                                     start=True, stop=False)
                    nc.tensor.matmul(pq[mt][:], elr[0][:, ms2], upv[0][:, 1, :],
                                     start=False, stop=False)
                    # src tile 1 contracts [0:127]: row 127 is the pad node,
                    # which carries phantom mass under the u8 encoding
                    # (exp(0)=1 pad column) and must not propagate
                    nc.tensor.matmul(pq[mt][:], elr[1][0:127, ms], upv[1][0:127, 0, :],
                                     start=False, stop=False)
                    nc.tensor.matmul(pq[mt][:], elr[1][0:127, ms2], upv[1][0:127, 1, :],
                                     start=False, stop=True)

            # ---- leaf block (once; pleaf pre-allocated as warm-up target) ----
            nc.tensor.matmul(pleaf[:], sacc_bf[0][:, 0, :], elr[0][:, 256:512],
                             start=True, stop=False)
            nc.tensor.matmul(pleaf[:], sacc_bf[0][:, 1, :], elr[0][:, 768:1024],
                             start=False, stop=False)
            nc.tensor.matmul(pleaf[:], sacc_bf[1][0:127, 0, :], elr[1][0:127, 256:512],
                             start=False, stop=False)
            nc.tensor.matmul(pleaf[:], sacc_bf[1][0:127, 1, :], elr[1][0:127, 768:1024],
                             start=False, stop=True)

            # ---- output ----
            o = work.tile([BH, 256], bf16, tag="o", name="o")
            nc.vector.tensor_copy(o[:], pleaf[:])
            nc.sync.dma_start(p_out[:, :], o[:])

    nc.finalize()
    return nc


def _get_program():
    if "nc" not in _CACHE:
        _CACHE["nc"] = _build_program()
    return _CACHE["nc"]


def _prep_inputs(x, W, bias, M_left, M_right):
    """Host-side shard + layout prep. Core c -> graph c//2, batch half c%2."""
    in_maps = []
    m2_g, wt_g, bias_g = [], [], []
    for g in range(G):
        m2 = np.zeros((256, 1024), np.float32)
        tl = M_left[g].T  # (255, 511): src-major
        tr = M_right[g].T
        for base, src in ((0, tl), (512, tr)):
            m2[0:255, base:base + 255] = src[:, 0:255]
            m2[0:255, base + 256:base + 512] = src[:, 255:511]
            m2[0:255, base + 255] = NEG
        m2_g.append(m2.astype(BF16))
        wt = np.zeros((512, 256), np.float32)
        wt[:, 0:255] = W[g].T
        wt_g.append(wt)
        bp = np.zeros((256,), np.float32)
        bp[0:255] = bias[g] * 0.5
        bias_g.append(np.ascontiguousarray(bp.reshape(2, 128).T))  # (128, 2)
    xt_h = [np.ascontiguousarray(x[h * BH:(h + 1) * BH].T) for h in range(2)]
    for c in range(NCORES):
        g, h = c // 2, c % 2
        wxc = np.concatenate([wt_g[g], xt_h[h]], axis=1)  # (512, 320)
        wxp = np.ascontiguousarray(
            wxc.reshape(4, 128, 320).transpose(1, 0, 2).reshape(128, 1280)
        ).astype(BF16)
        in_maps.append({
            "m2": m2_g[g], "wxp": wxp, "biasp": bias_g[g],
        })
    return in_maps


def _assemble(results):
    eps = np.float32(1e-5)
    ret = np.empty((B, L, G), np.float32)
    for c in range(NCORES):
        g, h = c // 2, c % 2
        ret[h * BH:(h + 1) * BH, :, g] = results[c]["out"].astype(np.float32)
    ret = np.where(ret > 0.0, ret, eps)
    ret = np.where(ret < 1.0, ret, np.float32(1.0) - eps)
    return ret.astype(np.float32)


def run_on_device(in_maps, trace=False, **kw):
    from concourse.bass_utils import run_bass_kernel_spmd
    nc = _get_program()
    return run_bass_kernel_spmd(nc, in_maps, list(range(NCORES)), trace=trace, **kw)


def kernel(x, W, bias, M_left, M_right):
    in_maps = _prep_inputs(
        np.asarray(x, np.float32), np.asarray(W, np.float32),
        np.asarray(bias, np.float32), np.asarray(M_left, np.float32),
        np.asarray(M_right, np.float32),
    )
    res = run_on_device(in_maps)
    return _assemble(res.results)
